# revision 12
# baseline (speedup 1.0000x reference)
"""KV-cache MHA Trainium2 kernel, 8-way tensor-parallel over heads.

Contract: kernel(**inputs) takes the FULL inputs (x, past_k, past_v, Wq, bq,
Wk, bk, Wv, bv, Wo, bo) and returns (out, K, V) exactly like the reference.
Internally: each of the 8 NeuronCores handles 2 heads (QKV projection columns,
attention, and its additive share of the output projection). The host sums the
8 partial out-projections and assembles the K/V caches.
"""

import sys

for _p in ("/opt/trn_rl_repo", "/root/.axon_site"):
    if _p not in sys.path:
        sys.path.insert(0, _p)

import math

import numpy as np

import concourse.bass as bass
import concourse.mybir as mybir
import concourse.tile as tile
from concourse import bacc
from concourse.bass_utils import run_bass_kernel_spmd
from concourse.masks import make_identity

# problem shape (hardcoded per spec)
B, S, D, H = 2, 2048, 2048, 16
HD = D // H  # 128
PAST = 2048
KL = PAST + S  # 4096
BS = B * S  # 4096
NCORES = 8
NH = H // NCORES  # 2 heads per core
HDC = NH * HD  # 256 head-dims per core
SCALE = 1.0 / math.sqrt(HD)

F32 = mybir.dt.float32
F32R = mybir.dt.float32r

# matmul input dtype: float32r runs the PE at full rate (vs 4 cyc/row fp32)
USE_F32R = True
MDT = F32R if USE_F32R else F32

ST = 256  # projection s-tile rows
NST = S // ST  # 8 s-tiles per batch
QT = 256  # attention q-tile
NQT = S // QT  # 8 q-tiles per (b,h)
NKC = KL // 128  # 32 k-chunks




def build_nc():
    nc = bacc.Bacc("TRN2", target_bir_lowering=False, debug=False)

    # per-core DRAM inputs
    x_f = nc.dram_tensor("x_f", [BS, D], F32, kind="ExternalInput").ap()
    wq = nc.dram_tensor("wq", [D, HDC], F32, kind="ExternalInput").ap()
    wk = nc.dram_tensor("wk", [D, HDC], F32, kind="ExternalInput").ap()
    wv = nc.dram_tensor("wv", [D, HDC], F32, kind="ExternalInput").ap()
    wo = nc.dram_tensor("wo", [HDC, D], F32, kind="ExternalInput").ap()
    bqTs = nc.dram_tensor("bqTs", [128, NH], F32, kind="ExternalInput").ap()
    bkT = nc.dram_tensor("bkT", [128, NH], F32, kind="ExternalInput").ap()
    bv_r = nc.dram_tensor("bv_r", [1, HDC], F32, kind="ExternalInput").ap()
    pkT = nc.dram_tensor("pkT", [B, NH, HD, PAST], F32, kind="ExternalInput").ap()
    pv = nc.dram_tensor("pv", [B, NH, PAST, HD], F32, kind="ExternalInput").ap()
    msk = nc.dram_tensor("msk", [128, 2, QT], F32, kind="ExternalInput").ap()

    outp = nc.dram_tensor("outp", [BS, D], F32, kind="ExternalOutput").ap()
    kn = nc.dram_tensor("kn", [B, NH, S, HD], F32, kind="ExternalOutput").ap()
    vn = nc.dram_tensor("vn", [B, NH, S, HD], F32, kind="ExternalOutput").ap()

    with tile.TileContext(nc) as tc:
        build_tile_kernel(
            tc, x_f, wq, wk, wv, wo, bqTs, bkT, bv_r, pkT, pv, msk, outp, kn, vn
        )
    nc.compile()
    return nc


def build_tile_kernel(
    tc, x_f, wq, wk, wv, wo, bqTs, bkT, bv_r, pkT, pv, msk, outp, kn, vn
):
    nc = tc.nc
    Act = mybir.ActivationFunctionType

    persist = tc.alloc_tile_pool(name="persist", bufs=1)
    perb = tc.alloc_tile_pool(name="perb", bufs=1)

    # constants
    ident = persist.tile([128, 128], F32, tag="ident")
    make_identity(nc, ident)
    ident_r = persist.tile([128, 128], MDT, tag="identr")
    nc.vector.tensor_copy(ident_r, ident)
    ones_f32 = persist.tile([128, 128], F32, tag="ones32")
    nc.vector.memset(ones_f32, 1.0)
    ones_sb = persist.tile([128, 128], MDT, tag="ones")
    nc.vector.tensor_copy(ones_sb, ones_f32)
    msk_sb = persist.tile([128, 2, QT], F32, tag="msk")
    nc.sync.dma_start(msk_sb, msk)
    bq_sb = persist.tile([128, NH], F32, tag="bq")
    nc.sync.dma_start(bq_sb, bqTs)
    bk_sb = persist.tile([128, NH], F32, tag="bk")
    nc.sync.dma_start(bk_sb, bkT)
    bv_sb = persist.tile([128, HDC], F32, tag="bv")
    nc.sync.dma_start(
        bv_sb, bass.AP(tensor=bv_r.tensor, offset=bv_r.offset, ap=[[0, 128], [1, HDC]])
    )
    # weights resident (stream wo per out-proj phase)
    wq_sb = persist.tile([128, D // 128, HDC], MDT, tag="wq")
    nc.sync.dma_start(wq_sb, wq.rearrange("(c p) n -> p c n", p=128).bitcast(MDT))
    wk_sb = persist.tile([128, D // 128, HDC], MDT, tag="wk")
    nc.sync.dma_start(wk_sb, wk.rearrange("(c p) n -> p c n", p=128).bitcast(MDT))
    wv_sb = persist.tile([128, D // 128, HDC], MDT, tag="wv")
    nc.sync.dma_start(wv_sb, wv.rearrange("(c p) n -> p c n", p=128).bitcast(MDT))

    for b in range(B):
        # ---------------- phase P: projections for batch b ----------------
        QT_sb = perb.tile([128, NH, S], MDT, tag="QT")  # [hd, h, q]
        KT_sb = perb.tile([128, NH, S], MDT, tag="KT")  # [hd, h, k_new]
        v_sb = perb.tile([128, S // 128, HDC], MDT, tag="V")  # [k%128, k//128, h*hd]

        with (
            tc.tile_pool(name="pstage", bufs=2) as pstage,
            tc.tile_pool(name="pxin", bufs=2) as pxin,
            tc.tile_pool(name="pkout", bufs=3) as pkout,
            tc.tile_pool(name="ps_t", bufs=2, space="PSUM") as ps_t,
            tc.tile_pool(name="ps_mm", bufs=3, space="PSUM") as ps_mm,
        ):
            for st in range(NST):
                s0 = b * S + st * ST  # row into x_f / outp
                x_sb = pxin.tile([128, ST // 128, D], F32, tag="x")
                nc.sync.dma_start(
                    x_sb, x_f[s0 : s0 + ST, :].rearrange("(c p) d -> p c d", p=128)
                )
                xT = pstage.tile([128, D // 128, ST], MDT, tag="xT")
                for c2 in range(ST // 128):
                    for dc in range(D // 128):
                        pst = ps_t.tile([128, 128], F32, tag="pst")
                        nc.tensor.transpose(
                            pst, x_sb[:, c2, dc * 128 : (dc + 1) * 128], ident
                        )
                        dst = xT[:, dc, c2 * 128 : (c2 + 1) * 128]
                        if (dc + c2) % 2 == 0:
                            nc.vector.tensor_copy(dst, pst)
                        else:
                            nc.scalar.copy(dst, pst)

                # Q^T and K^T: [hd(m), q] = sum_d w[d, hd] x^T[d, q]
                for m in range(NH):
                    psq = ps_mm.tile([128, ST], F32, tag="pmm")
                    for dc in range(D // 128):
                        nc.tensor.matmul(
                            psq,
                            wq_sb[:, dc, m * 128 : (m + 1) * 128],
                            xT[:, dc, :],
                            start=(dc == 0),
                            stop=(dc == D // 128 - 1),
                        )
                    nc.vector.tensor_scalar(
                        QT_sb[:, m, st * ST : (st + 1) * ST],
                        psq,
                        SCALE,
                        bq_sb[:, m : m + 1],
                        mybir.AluOpType.mult,
                        mybir.AluOpType.add,
                    )
                for m in range(NH):
                    psk = ps_mm.tile([128, ST], F32, tag="pmm")
                    for dc in range(D // 128):
                        nc.tensor.matmul(
                            psk,
                            wk_sb[:, dc, m * 128 : (m + 1) * 128],
                            xT[:, dc, :],
                            start=(dc == 0),
                            stop=(dc == D // 128 - 1),
                        )
                    kslice = KT_sb[:, m, st * ST : (st + 1) * ST]
                    nc.vector.tensor_scalar(
                        kslice,
                        psk,
                        bk_sb[:, m : m + 1],
                        None,
                        mybir.AluOpType.add,
                    )
                    # K cache output needs natural [s, hd]: transpose back
                    for sc2 in range(ST // 128):
                        pst2 = ps_t.tile([128, 128], MDT, tag="pst")
                        nc.tensor.transpose(
                            pst2,
                            kslice[:, sc2 * 128 : (sc2 + 1) * 128],
                            ident_r,
                        )
                        ko = pkout.tile([128, 128], F32, tag="ko")
                        nc.vector.tensor_copy(ko, pst2)
                        sl = st * ST + sc2 * 128
                        nc.sync.dma_start(kn[b, m, sl : sl + 128, :], ko)

                # V natural: [s(m), hd] = sum_d x^T[d, s] wv[d, hd]
                for sc2 in range(ST // 128):
                    psv = ps_mm.tile([128, HDC], F32, tag="pmm")
                    for dc in range(D // 128):
                        nc.tensor.matmul(
                            psv,
                            xT[:, dc, sc2 * 128 : (sc2 + 1) * 128],
                            wv_sb[:, dc, :],
                            start=(dc == 0),
                            stop=(dc == D // 128 - 1),
                        )
                    vdst = v_sb[:, st * (ST // 128) + sc2, :]
                    nc.vector.tensor_add(vdst, psv, bv_sb)
                    sl = st * ST + sc2 * 128
                    for m in range(NH):
                        nc.sync.dma_start(
                            vn[b, m, sl : sl + 128, :],
                            vdst[:, m * 128 : (m + 1) * 128].bitcast(F32),
                        )

        # ---------------- phase A: attention for batch b ----------------
        ctxT_sb = perb.tile([128, NH, S], MDT, tag="ctxT")  # [hd, h, q]
        with (
            tc.tile_pool(name="apk", bufs=2) as apk,
            tc.tile_pool(name="apv", bufs=2) as apv,
            tc.tile_pool(name="aexp", bufs=8) as aexp,
            tc.tile_pool(name="arc", bufs=2) as arc,
            tc.tile_pool(name="ps_s", bufs=3, space="PSUM") as ps_s,
            tc.tile_pool(name="ps_ct", bufs=2, space="PSUM") as ps_ct,
            tc.tile_pool(name="ps_r", bufs=2, space="PSUM") as ps_r,
        ):
            for h in range(NH):
                pkT_sb = apk.tile([128, PAST], MDT, tag="pkT")
                nc.sync.dma_start(pkT_sb, pkT[b, h].bitcast(MDT))
                pv_sb = apv.tile([128, PAST // 128, HD], MDT, tag="pv")
                nc.sync.dma_start(pv_sb, pv[b, h].rearrange("(c p) e -> p c e", p=128).bitcast(MDT))
                for qt in range(NQT):
                    q0 = qt * QT
                    kc_end = min(NKC, 2 * qt + (QT // 128) + 16)
                    ct = ps_ct.tile([128, QT], F32, tag="ct")
                    rs = ps_r.tile([128, QT], F32, tag="rs")
                    for kc in range(kc_end):
                        if kc < PAST // 128:
                            lk = pkT_sb[:, kc * 128 : (kc + 1) * 128]
                            lv = pv_sb[:, kc, :]
                        else:
                            k0 = (kc - PAST // 128) * 128
                            lk = KT_sb[:, h, k0 : k0 + 128]
                            lv = v_sb[:, k0 // 128, h * 128 : (h + 1) * 128]
                        sc = ps_s.tile([128, QT], F32, tag="sc")
                        nc.tensor.matmul(
                            sc,
                            lk,
                            QT_sb[:, h, q0 : q0 + QT],
                            start=True,
                            stop=True,
                        )
                        dk = kc - (2 * qt + 16)
                        if dk == 0:
                            nc.vector.tensor_add(sc, sc, msk_sb[:, 0, :])
                        elif dk == 1:
                            nc.vector.tensor_add(sc, sc, msk_sb[:, 1, :])
                        et = aexp.tile([128, QT], MDT, tag="et")
                        nc.scalar.activation(et, sc, Act.Exp)
                        nc.tensor.matmul(
                            ct,
                            lv,
                            et,
                            start=(kc == 0),
                            stop=(kc == kc_end - 1),
                        )
                        nc.tensor.matmul(
                            rs,
                            ones_sb,
                            et,
                            start=(kc == 0),
                            stop=(kc == kc_end - 1),
                        )
                    rc = arc.tile([128, QT], F32, tag="rc")
                    nc.vector.reciprocal(rc, rs)
                    nc.vector.tensor_mul(ctxT_sb[:, h, q0 : q0 + QT], ct, rc)

        # ---------------- phase O: out projection partial for batch b ----------
        with (
            tc.tile_pool(name="owo", bufs=1) as owo,
            tc.tile_pool(name="oout", bufs=3) as oout,
            tc.tile_pool(name="ps_o", bufs=3, space="PSUM") as ps_o,
        ):
            wo_sb = owo.tile([128, NH, D], MDT, tag="wo")
            nc.sync.dma_start(wo_sb, wo.rearrange("(m p) d -> p m d", p=128).bitcast(MDT))
            for sc_i in range(S // 128):
                for dt in range(D // 512):
                    po = ps_o.tile([128, 512], F32, tag="po")
                    for m in range(NH):
                        nc.tensor.matmul(
                            po,
                            ctxT_sb[:, m, sc_i * 128 : (sc_i + 1) * 128],
                            wo_sb[:, m, dt * 512 : (dt + 1) * 512],
                            start=(m == 0),
                            stop=(m == NH - 1),
                        )
                    ob = oout.tile([128, 512], F32, tag="ob")
                    if (sc_i + dt) % 2 == 0:
                        nc.vector.tensor_copy(ob, po)
                    else:
                        nc.scalar.copy(ob, po)
                    nc.sync.dma_start(
                        outp[
                            b * S + sc_i * 128 : b * S + (sc_i + 1) * 128,
                            dt * 512 : (dt + 1) * 512,
                        ],
                        ob,
                    )

    perb.release()
    persist.release()


_NC_CACHE = {}


def get_nc():
    if "nc" not in _NC_CACHE:
        _NC_CACHE["nc"] = build_nc()
    return _NC_CACHE["nc"]


def make_in_maps(x, past_k, past_v, Wq, bq, Wk, bk, Wv, bv, Wo, bo):
    x_f = np.ascontiguousarray(np.asarray(x, np.float32).reshape(BS, D))
    Wq, Wk, Wv, Wo = (np.asarray(a, np.float32) for a in (Wq, Wk, Wv, Wo))
    bq, bk, bv = (np.asarray(a, np.float32) for a in (bq, bk, bv))
    past_k = np.asarray(past_k, np.float32)
    past_v = np.asarray(past_v, np.float32)

    # two causal boundary mask patterns (keys on partitions, queries on free)
    dk = np.arange(128)[:, None]
    dq = np.arange(QT)[None, :]
    msk = np.zeros((128, 2, QT), np.float32)
    msk[:, 0, :] = np.where(dk <= dq, 0.0, -1e30)
    msk[:, 1, :] = np.where(dk <= dq - 128, 0.0, -1e30)

    in_maps = []
    for c in range(NCORES):
        h0 = c * NH
        sl = slice(h0 * HD, (h0 + NH) * HD)
        in_maps.append(
            {
                "x_f": x_f,
                "wq": np.ascontiguousarray(Wq[:, sl]),
                "wk": np.ascontiguousarray(Wk[:, sl]),
                "wv": np.ascontiguousarray(Wv[:, sl]),
                "wo": np.ascontiguousarray(Wo[sl, :]),
                "bqTs": np.ascontiguousarray(
                    (bq[sl] * SCALE).reshape(NH, HD).T
                ),
                "bkT": np.ascontiguousarray(bk[sl].reshape(NH, HD).T),
                "bv_r": np.ascontiguousarray(bv[sl].reshape(1, HDC)),
                "pkT": np.ascontiguousarray(
                    past_k[:, h0 : h0 + NH].transpose(0, 1, 3, 2)
                ),
                "pv": np.ascontiguousarray(past_v[:, h0 : h0 + NH]),
                "msk": msk,
            }
        )
    return in_maps


def assemble(results, past_k, past_v, bo):
    out = np.zeros((BS, D), np.float32)
    for c in range(NCORES):
        out += results[c]["outp"]
    out += np.asarray(bo, np.float32)[None, :]
    out = out.reshape(B, S, D)

    K = np.empty((B, H, KL, HD), np.float32)
    V = np.empty((B, H, KL, HD), np.float32)
    K[:, :, :PAST] = past_k
    V[:, :, :PAST] = past_v
    for c in range(NCORES):
        h0 = c * NH
        K[:, h0 : h0 + NH, PAST:] = results[c]["kn"]
        V[:, h0 : h0 + NH, PAST:] = results[c]["vn"]
    return out, K, V


def kernel(x, past_k, past_v, Wq, bq, Wk, bk, Wv, bv, Wo, bo):
    nc = get_nc()
    in_maps = make_in_maps(x, past_k, past_v, Wq, bq, Wk, bk, Wv, bv, Wo, bo)
    res = run_bass_kernel_spmd(nc, in_maps, core_ids=list(range(NCORES)))
    return assemble(
        res.results, np.asarray(past_k, np.float32), np.asarray(past_v, np.float32), bo
    )


# revision 13
# speedup vs baseline: 1.0280x; 1.0280x over previous
"""KV-cache MHA Trainium2 kernel, 8-way tensor-parallel over heads.

Contract: kernel(**inputs) takes the FULL inputs (x, past_k, past_v, Wq, bq,
Wk, bk, Wv, bv, Wo, bo) and returns (out, K, V) exactly like the reference.
Internally: each of the 8 NeuronCores handles 2 heads (QKV projection columns,
attention, and its additive share of the output projection). The host sums the
8 partial out-projections and assembles the K/V caches.
"""

import sys

for _p in ("/opt/trn_rl_repo", "/root/.axon_site"):
    if _p not in sys.path:
        sys.path.insert(0, _p)

import math

import numpy as np

import concourse.bass as bass
import concourse.mybir as mybir
import concourse.tile as tile
from concourse import bacc
from concourse.bass_utils import run_bass_kernel_spmd
from concourse.masks import make_identity

# problem shape (hardcoded per spec)
B, S, D, H = 2, 2048, 2048, 16
HD = D // H  # 128
PAST = 2048
KL = PAST + S  # 4096
BS = B * S  # 4096
NCORES = 8
NH = H // NCORES  # 2 heads per core
HDC = NH * HD  # 256 head-dims per core
SCALE = 1.0 / math.sqrt(HD)

F32 = mybir.dt.float32
F32R = mybir.dt.float32r

# matmul input dtype: float32r runs the PE at full rate (vs 4 cyc/row fp32)
USE_F32R = True
MDT = F32R if USE_F32R else F32

ST = 256  # projection s-tile rows
NST = S // ST  # 8 s-tiles per batch
QT = 512  # attention q-tile
NQT = S // QT  # 8 q-tiles per (b,h)
NKC = KL // 128  # 32 k-chunks




def build_nc():
    nc = bacc.Bacc("TRN2", target_bir_lowering=False, debug=False)

    # per-core DRAM inputs
    x_f = nc.dram_tensor("x_f", [BS, D], F32, kind="ExternalInput").ap()
    wq = nc.dram_tensor("wq", [D, HDC], F32, kind="ExternalInput").ap()
    wk = nc.dram_tensor("wk", [D, HDC], F32, kind="ExternalInput").ap()
    wv = nc.dram_tensor("wv", [D, HDC], F32, kind="ExternalInput").ap()
    wo = nc.dram_tensor("wo", [HDC, D], F32, kind="ExternalInput").ap()
    bqTs = nc.dram_tensor("bqTs", [128, NH], F32, kind="ExternalInput").ap()
    bkT = nc.dram_tensor("bkT", [128, NH], F32, kind="ExternalInput").ap()
    bv_r = nc.dram_tensor("bv_r", [1, HDC], F32, kind="ExternalInput").ap()
    pkT = nc.dram_tensor("pkT", [B, NH, HD, PAST], F32, kind="ExternalInput").ap()
    pv = nc.dram_tensor("pv", [B, NH, PAST, HD], F32, kind="ExternalInput").ap()
    msk = nc.dram_tensor("msk", [128, 4, QT], F32, kind="ExternalInput").ap()

    outp = nc.dram_tensor("outp", [BS, D], F32, kind="ExternalOutput").ap()
    kn = nc.dram_tensor("kn", [B, NH, S, HD], F32, kind="ExternalOutput").ap()
    vn = nc.dram_tensor("vn", [B, NH, S, HD], F32, kind="ExternalOutput").ap()

    with tile.TileContext(nc) as tc:
        build_tile_kernel(
            tc, x_f, wq, wk, wv, wo, bqTs, bkT, bv_r, pkT, pv, msk, outp, kn, vn
        )
    nc.compile()
    return nc


def build_tile_kernel(
    tc, x_f, wq, wk, wv, wo, bqTs, bkT, bv_r, pkT, pv, msk, outp, kn, vn
):
    nc = tc.nc
    Act = mybir.ActivationFunctionType

    persist = tc.alloc_tile_pool(name="persist", bufs=1)
    perb = tc.alloc_tile_pool(name="perb", bufs=1)

    # constants
    ident = persist.tile([128, 128], F32, tag="ident")
    make_identity(nc, ident)
    ident_r = persist.tile([128, 128], MDT, tag="identr")
    nc.vector.tensor_copy(ident_r, ident)
    ones_f32 = persist.tile([128, 128], F32, tag="ones32")
    nc.vector.memset(ones_f32, 1.0)
    ones_sb = persist.tile([128, 128], MDT, tag="ones")
    nc.vector.tensor_copy(ones_sb, ones_f32)
    msk_sb = persist.tile([128, 4, QT], F32, tag="msk")
    nc.sync.dma_start(msk_sb, msk)
    bq_sb = persist.tile([128, NH], F32, tag="bq")
    nc.sync.dma_start(bq_sb, bqTs)
    bk_sb = persist.tile([128, NH], F32, tag="bk")
    nc.sync.dma_start(bk_sb, bkT)
    bv_sb = persist.tile([128, HDC], F32, tag="bv")
    nc.sync.dma_start(
        bv_sb, bass.AP(tensor=bv_r.tensor, offset=bv_r.offset, ap=[[0, 128], [1, HDC]])
    )
    # weights resident (stream wo per out-proj phase)
    wq_sb = persist.tile([128, D // 128, HDC], MDT, tag="wq")
    nc.sync.dma_start(wq_sb, wq.rearrange("(c p) n -> p c n", p=128).bitcast(MDT))
    wk_sb = persist.tile([128, D // 128, HDC], MDT, tag="wk")
    nc.sync.dma_start(wk_sb, wk.rearrange("(c p) n -> p c n", p=128).bitcast(MDT))
    wv_sb = persist.tile([128, D // 128, HDC], MDT, tag="wv")
    nc.sync.dma_start(wv_sb, wv.rearrange("(c p) n -> p c n", p=128).bitcast(MDT))

    for b in range(B):
        # ---------------- phase P: projections for batch b ----------------
        QT_sb = perb.tile([128, NH, S], MDT, tag="QT")  # [hd, h, q]
        KT_sb = perb.tile([128, NH, S], MDT, tag="KT")  # [hd, h, k_new]
        v_sb = perb.tile([128, S // 128, HDC], MDT, tag="V")  # [k%128, k//128, h*hd]

        with (
            tc.tile_pool(name="pstage", bufs=2) as pstage,
            tc.tile_pool(name="pxin", bufs=2) as pxin,
            tc.tile_pool(name="pkout", bufs=3) as pkout,
            tc.tile_pool(name="ps_t", bufs=2, space="PSUM") as ps_t,
            tc.tile_pool(name="ps_mm", bufs=3, space="PSUM") as ps_mm,
        ):
            for st in range(NST):
                s0 = b * S + st * ST  # row into x_f / outp
                x_sb = pxin.tile([128, ST // 128, D], F32, tag="x")
                nc.sync.dma_start(
                    x_sb, x_f[s0 : s0 + ST, :].rearrange("(c p) d -> p c d", p=128)
                )
                xT = pstage.tile([128, D // 128, ST], MDT, tag="xT")
                for c2 in range(ST // 128):
                    for dc in range(D // 128):
                        pst = ps_t.tile([128, 128], F32, tag="pst")
                        nc.tensor.transpose(
                            pst, x_sb[:, c2, dc * 128 : (dc + 1) * 128], ident
                        )
                        dst = xT[:, dc, c2 * 128 : (c2 + 1) * 128]
                        if (dc + c2) % 2 == 0:
                            nc.vector.tensor_copy(dst, pst)
                        else:
                            nc.scalar.copy(dst, pst)

                # Q^T and K^T: [hd(m), q] = sum_d w[d, hd] x^T[d, q]
                for m in range(NH):
                    psq = ps_mm.tile([128, ST], F32, tag="pmm")
                    for dc in range(D // 128):
                        nc.tensor.matmul(
                            psq,
                            wq_sb[:, dc, m * 128 : (m + 1) * 128],
                            xT[:, dc, :],
                            start=(dc == 0),
                            stop=(dc == D // 128 - 1),
                        )
                    nc.vector.tensor_scalar(
                        QT_sb[:, m, st * ST : (st + 1) * ST],
                        psq,
                        SCALE,
                        bq_sb[:, m : m + 1],
                        mybir.AluOpType.mult,
                        mybir.AluOpType.add,
                    )
                for m in range(NH):
                    psk = ps_mm.tile([128, ST], F32, tag="pmm")
                    for dc in range(D // 128):
                        nc.tensor.matmul(
                            psk,
                            wk_sb[:, dc, m * 128 : (m + 1) * 128],
                            xT[:, dc, :],
                            start=(dc == 0),
                            stop=(dc == D // 128 - 1),
                        )
                    kslice = KT_sb[:, m, st * ST : (st + 1) * ST]
                    nc.vector.tensor_scalar(
                        kslice,
                        psk,
                        bk_sb[:, m : m + 1],
                        None,
                        mybir.AluOpType.add,
                    )
                    # K cache output needs natural [s, hd]: transpose back
                    for sc2 in range(ST // 128):
                        pst2 = ps_t.tile([128, 128], MDT, tag="pst")
                        nc.tensor.transpose(
                            pst2,
                            kslice[:, sc2 * 128 : (sc2 + 1) * 128],
                            ident_r,
                        )
                        ko = pkout.tile([128, 128], F32, tag="ko")
                        nc.vector.tensor_copy(ko, pst2)
                        sl = st * ST + sc2 * 128
                        nc.sync.dma_start(kn[b, m, sl : sl + 128, :], ko)

                # V natural: [s(m), hd] = sum_d x^T[d, s] wv[d, hd]
                for sc2 in range(ST // 128):
                    psv = ps_mm.tile([128, HDC], F32, tag="pmm")
                    for dc in range(D // 128):
                        nc.tensor.matmul(
                            psv,
                            xT[:, dc, sc2 * 128 : (sc2 + 1) * 128],
                            wv_sb[:, dc, :],
                            start=(dc == 0),
                            stop=(dc == D // 128 - 1),
                        )
                    vdst = v_sb[:, st * (ST // 128) + sc2, :]
                    nc.vector.tensor_add(vdst, psv, bv_sb)
                    sl = st * ST + sc2 * 128
                    for m in range(NH):
                        nc.sync.dma_start(
                            vn[b, m, sl : sl + 128, :],
                            vdst[:, m * 128 : (m + 1) * 128].bitcast(F32),
                        )

        # ---------------- phase A: attention for batch b ----------------
        ctxT_sb = perb.tile([128, NH, S], MDT, tag="ctxT")  # [hd, h, q]
        with (
            tc.tile_pool(name="apk", bufs=2) as apk,
            tc.tile_pool(name="apv", bufs=2) as apv,
            tc.tile_pool(name="aexp", bufs=8) as aexp,
            tc.tile_pool(name="arc", bufs=2) as arc,
            tc.tile_pool(name="ps_s", bufs=3, space="PSUM") as ps_s,
            tc.tile_pool(name="ps_ct", bufs=2, space="PSUM") as ps_ct,
            tc.tile_pool(name="ps_r", bufs=2, space="PSUM") as ps_r,
        ):
            for h in range(NH):
                pkT_sb = apk.tile([128, PAST], MDT, tag="pkT")
                nc.sync.dma_start(pkT_sb, pkT[b, h].bitcast(MDT))
                pv_sb = apv.tile([128, PAST // 128, HD], MDT, tag="pv")
                nc.sync.dma_start(pv_sb, pv[b, h].rearrange("(c p) e -> p c e", p=128).bitcast(MDT))
                for qt in range(NQT):
                    q0 = qt * QT
                    kc_end = min(NKC, (QT // 128) * qt + (QT // 128) + 16)
                    ct = ps_ct.tile([128, QT], F32, tag="ct")
                    rs = ps_r.tile([128, QT], F32, tag="rs")
                    for kc in range(kc_end):
                        if kc < PAST // 128:
                            lk = pkT_sb[:, kc * 128 : (kc + 1) * 128]
                            lv = pv_sb[:, kc, :]
                        else:
                            k0 = (kc - PAST // 128) * 128
                            lk = KT_sb[:, h, k0 : k0 + 128]
                            lv = v_sb[:, k0 // 128, h * 128 : (h + 1) * 128]
                        sc = ps_s.tile([128, QT], F32, tag="sc")
                        nc.tensor.matmul(
                            sc,
                            lk,
                            QT_sb[:, h, q0 : q0 + QT],
                            start=True,
                            stop=True,
                        )
                        dk = kc - ((QT // 128) * qt + 16)
                        if 0 <= dk < QT // 128:
                            nc.vector.tensor_add(sc, sc, msk_sb[:, dk, :])
                        et = aexp.tile([128, QT], MDT, tag="et")
                        nc.scalar.activation(et, sc, Act.Exp)
                        nc.tensor.matmul(
                            ct,
                            lv,
                            et,
                            start=(kc == 0),
                            stop=(kc == kc_end - 1),
                        )
                        nc.tensor.matmul(
                            rs,
                            ones_sb,
                            et,
                            start=(kc == 0),
                            stop=(kc == kc_end - 1),
                        )
                    rc = arc.tile([128, QT], F32, tag="rc")
                    nc.vector.reciprocal(rc, rs)
                    nc.vector.tensor_mul(ctxT_sb[:, h, q0 : q0 + QT], ct, rc)

        # ---------------- phase O: out projection partial for batch b ----------
        with (
            tc.tile_pool(name="owo", bufs=1) as owo,
            tc.tile_pool(name="oout", bufs=3) as oout,
            tc.tile_pool(name="ps_o", bufs=3, space="PSUM") as ps_o,
        ):
            wo_sb = owo.tile([128, NH, D], MDT, tag="wo")
            nc.sync.dma_start(wo_sb, wo.rearrange("(m p) d -> p m d", p=128).bitcast(MDT))
            for sc_i in range(S // 128):
                for dt in range(D // 512):
                    po = ps_o.tile([128, 512], F32, tag="po")
                    for m in range(NH):
                        nc.tensor.matmul(
                            po,
                            ctxT_sb[:, m, sc_i * 128 : (sc_i + 1) * 128],
                            wo_sb[:, m, dt * 512 : (dt + 1) * 512],
                            start=(m == 0),
                            stop=(m == NH - 1),
                        )
                    ob = oout.tile([128, 512], F32, tag="ob")
                    if (sc_i + dt) % 2 == 0:
                        nc.vector.tensor_copy(ob, po)
                    else:
                        nc.scalar.copy(ob, po)
                    nc.sync.dma_start(
                        outp[
                            b * S + sc_i * 128 : b * S + (sc_i + 1) * 128,
                            dt * 512 : (dt + 1) * 512,
                        ],
                        ob,
                    )

    perb.release()
    persist.release()


_NC_CACHE = {}


def get_nc():
    if "nc" not in _NC_CACHE:
        _NC_CACHE["nc"] = build_nc()
    return _NC_CACHE["nc"]


def make_in_maps(x, past_k, past_v, Wq, bq, Wk, bk, Wv, bv, Wo, bo):
    x_f = np.ascontiguousarray(np.asarray(x, np.float32).reshape(BS, D))
    Wq, Wk, Wv, Wo = (np.asarray(a, np.float32) for a in (Wq, Wk, Wv, Wo))
    bq, bk, bv = (np.asarray(a, np.float32) for a in (bq, bk, bv))
    past_k = np.asarray(past_k, np.float32)
    past_v = np.asarray(past_v, np.float32)

    # two causal boundary mask patterns (keys on partitions, queries on free)
    dk = np.arange(128)[:, None]
    dq = np.arange(QT)[None, :]
    msk = np.zeros((128, QT // 128, QT), np.float32)
    for j in range(QT // 128):
        msk[:, j, :] = np.where(dk <= dq - 128 * j, 0.0, -1e30)

    in_maps = []
    for c in range(NCORES):
        h0 = c * NH
        sl = slice(h0 * HD, (h0 + NH) * HD)
        in_maps.append(
            {
                "x_f": x_f,
                "wq": np.ascontiguousarray(Wq[:, sl]),
                "wk": np.ascontiguousarray(Wk[:, sl]),
                "wv": np.ascontiguousarray(Wv[:, sl]),
                "wo": np.ascontiguousarray(Wo[sl, :]),
                "bqTs": np.ascontiguousarray(
                    (bq[sl] * SCALE).reshape(NH, HD).T
                ),
                "bkT": np.ascontiguousarray(bk[sl].reshape(NH, HD).T),
                "bv_r": np.ascontiguousarray(bv[sl].reshape(1, HDC)),
                "pkT": np.ascontiguousarray(
                    past_k[:, h0 : h0 + NH].transpose(0, 1, 3, 2)
                ),
                "pv": np.ascontiguousarray(past_v[:, h0 : h0 + NH]),
                "msk": msk,
            }
        )
    return in_maps


def assemble(results, past_k, past_v, bo):
    out = np.zeros((BS, D), np.float32)
    for c in range(NCORES):
        out += results[c]["outp"]
    out += np.asarray(bo, np.float32)[None, :]
    out = out.reshape(B, S, D)

    K = np.empty((B, H, KL, HD), np.float32)
    V = np.empty((B, H, KL, HD), np.float32)
    K[:, :, :PAST] = past_k
    V[:, :, :PAST] = past_v
    for c in range(NCORES):
        h0 = c * NH
        K[:, h0 : h0 + NH, PAST:] = results[c]["kn"]
        V[:, h0 : h0 + NH, PAST:] = results[c]["vn"]
    return out, K, V


def kernel(x, past_k, past_v, Wq, bq, Wk, bk, Wv, bv, Wo, bo):
    nc = get_nc()
    in_maps = make_in_maps(x, past_k, past_v, Wq, bq, Wk, bk, Wv, bv, Wo, bo)
    res = run_bass_kernel_spmd(nc, in_maps, core_ids=list(range(NCORES)))
    return assemble(
        res.results, np.asarray(past_k, np.float32), np.asarray(past_v, np.float32), bo
    )
